# revision 1
# baseline (speedup 1.0000x reference)
"""Trainium2 Bass kernel for GQA attention block (B=2, S=2048, H=2048, NH=32, NKV=8, HD=64).

Sharding: 8 cores = data-parallel over batch (2) x tensor-parallel over heads (4).
Each core computes the qkv projection for its 8 q-heads / 2 kv-heads, RoPE,
causal GQA attention, and a partial o-projection (its 512 rows of w_o). The
host sums the 4 partial outputs per batch.

v3, power/throttle driven: the chip duty-cycles the PE to 4/8 when the
aggregate engine+DMA activity is high (measured: concurrent DVE work drops the
PE utilization limit from 0.92 to 0.67), so beyond scheduling, this version
minimizes DVE-op count and DMA descriptor count:
  - RoPE swap is one DVE stream_shuffle per job (host column permutation puts
    the rotate-half partner at p^16 within each 32-partition quadrant).
  - The softmax denominator rides ROW 0 of the PV output (ones column first
    in v, v at cols 64:128), so the reciprocal reads PSUM partition 0
    directly - no den copies; one [128,1024] PV tile per head-pass halves
    the reciprocal/broadcast/normalize op count.
  - Phase 1: weight-stationary s-pair streaming (each wq chunk loaded once
    for 4x512 cols); x lands as per-h DMAs interleaved across both pairs;
    feature order k, q0..q3, v.
  - Phase 2 pass A (q 0:1024) for all heads, then pass B with o-projection
    (qb, nch) pieces interleaved after every few attention jobs to fill the
    PE idle created by exp (ACT) waits; o-proj output batched per qb into
    one DMA.
  - Output fp16 (host sums partials in fp32).
"""

import sys

if "/opt/trn_rl_repo" not in sys.path:
    sys.path.insert(0, "/opt/trn_rl_repo")

import numpy as np

import concourse.bass as bass
import concourse.mybir as mybir
import concourse.tile as tile
from concourse import bacc
from concourse.bass_utils import run_bass_kernel_spmd

P = 128
S = 2048
H = 2048
NH = 32
NKV = 8
HD = 64
GROUPS = NH // NKV  # 4
NHL = 8   # local q heads per core
NKVL = 2  # local kv heads per core
FQ = NHL * HD   # 512
F = FQ + 2 * NKVL * HD  # 768
NKB = S // P    # 16 key blocks
ROPE_BASE = 10000.0

F32 = mybir.dt.float32
F16 = mybir.dt.float16

SWAP16 = [i ^ 16 for i in range(32)]  # rotate-half partner within quadrant


def build_bass():
    nc = bacc.Bacc("TRN2", num_devices=8)

    xT = nc.declare_dram_parameter("xT", [H, S], F16, isOutput=False)
    wqkv = nc.declare_dram_parameter("wqkv", [H, F], F16, isOutput=False)
    wo = nc.declare_dram_parameter("wo", [FQ, H], F16, isOutput=False)
    cosx = nc.declare_dram_parameter("cosx", [P, S], F16, isOutput=False)
    sinx = nc.declare_dram_parameter("sinx", [P, S], F16, isOutput=False)
    tri = nc.declare_dram_parameter("tri", [P, P], F16, isOutput=False)
    idn = nc.declare_dram_parameter("idn", [P, 64], F16, isOutput=False)
    out = nc.declare_dram_parameter("out", [S, H], F16, isOutput=True)

    with tile.TileContext(nc) as tc:
        with (
            tc.tile_pool(name="const", bufs=1) as const,
            tc.tile_pool(name="wq", bufs=1) as wqp,
            tc.tile_pool(name="qkvT", bufs=1) as qkvp,
            tc.tile_pool(name="vsb", bufs=1) as vsbp,
            tc.tile_pool(name="attnT", bufs=1) as attp,
            tc.tile_pool(name="wop", bufs=1) as wop,
        ):
            tri_sb = const.tile([P, P], F16)
            cos_sb = const.tile([P, S], F16)
            sin_sb = const.tile([P, S], F16)
            idn_sb = const.tile([P, 64], F16)
            wq_all = wqp.tile([P, H // P, F], F16)
            wo_all = wop.tile([P, 4, H], F16)

            # weight/table queue: tri first (warmup dep), then k+v weight
            # columns, tables, q weight columns, wo last (needed ~150us in).
            wq_re = wqkv.ap().rearrange("(ho p) f -> p ho f", p=P)
            nc.gpsimd.dma_start(out=tri_sb, in_=tri.ap())
            nc.gpsimd.dma_start(out=wq_all[:, :, 0:128], in_=wq_re[:, :, 0:128])
            nc.gpsimd.dma_start(out=wq_all[:, :, 256:384], in_=wq_re[:, :, 256:384])
            nc.gpsimd.dma_start(out=idn_sb, in_=idn.ap())
            for sl in (slice(0, 1024), slice(1024, 2048)):
                nc.gpsimd.dma_start(out=cos_sb[:, sl], in_=cosx.ap()[:, sl])
                nc.gpsimd.dma_start(out=sin_sb[:, sl], in_=sinx.ap()[:, sl])
            nc.gpsimd.dma_start(out=wq_all[:, :, 384:F], in_=wq_re[:, :, 384:F])
            nc.gpsimd.dma_start(out=wq_all[:, :, 128:256], in_=wq_re[:, :, 128:256])
            nc.gpsimd.dma_start(
                out=wo_all, in_=wo.ap().rearrange("(c p) n -> p c n", p=P)
            )

            qT_sb = [qkvp.tile([P, S], F16, tag=f"qT{c}", name=f"qT{c}") for c in range(4)]
            kT_rep = [qkvp.tile([P, S], F16, tag=f"kT{h}", name=f"kT{h}") for h in range(NKVL)]
            # v layout: col 0 = ones (softmax denominator rides PV row 0,
            # readable by the partition-0-based reciprocal), cols 1:64 zero,
            # cols 64:128 = v dims (64-aligned partition range in PV output).
            v_sb = [
                [vsbp.tile([P, P], F16, tag=f"v{hv}_{kb}", name=f"v{hv}_{kb}") for kb in range(NKB)]
                for hv in range(NKVL)
            ]
            attnT_sb = [attp.tile([P, S], F16, tag=f"at{c}", name=f"at{c}") for c in range(4)]

            for hv in range(NKVL):
                for kb in range(NKB):
                    nc.vector.memset(v_sb[hv][kb][:, 0:64], 0.0)
                    nc.vector.memset(v_sb[hv][kb][:, 0:1], 1.0)

            # PE warmup while input DMA streams: ramp the clock gate.
            with tc.tile_pool(name="wup", bufs=1, space="PSUM") as wupp:
                wup = wupp.tile([P, P], F32)
                for _ in range(16):
                    nc.tensor.matmul(wup, lhsT=tri_sb, rhs=tri_sb, start=True, stop=True)

            # ---------------- Phase 1: qkv^T = wqkv^T @ x^T, RoPE, v transpose
            # x lands as two 1024-col s-pair tiles; each weight chunk [h, f]
            # is loaded once and streams both pairs back-to-back (halves the
            # PE weight-switch tax). RoPE features first (k, q0..q3), v last.
            SCH = 1024
            xT_re = xT.ap().rearrange("(ho p) s -> p ho s", p=P)
            with tc.tile_pool(name="xw", bufs=1) as xw:
                xts = [
                    xw.tile([P, H // P, SCH], F16, tag=f"x{s}", name=f"x{s}")
                    for s in range(S // SCH)
                ]
                # interleave per-h transfers across both s-pairs so the
                # pair-interleaved k matmuls are never DMA-starved
                for h in range(H // P):
                    for s in range(S // SCH):
                        ssl = slice(s * SCH, (s + 1) * SCH)
                        nc.sync.dma_start(out=xts[s][:, h, :], in_=xT_re[:, h, ssl])

                def rope_feature(t, f, pr, ssl):
                    tsw = rtmp.tile([P, SCH], F32, tag="tsw", name="tsw")
                    nc.vector.stream_shuffle(tsw, t, SWAP16)
                    ta = rtmp.tile([P, SCH], F16, tag="ta", name="ta")
                    nc.vector.tensor_mul(ta, t, cos_sb[:, ssl])
                    ts2 = rtmp.tile([P, SCH], F16, tag="ts2", name="ts2")
                    nc.vector.tensor_mul(ts2, tsw, sin_sb[:, ssl])
                    tsw = ts2
                    if f == 0:
                        for hh in range(2):
                            si = slice(hh * 64, hh * 64 + 64)
                            nc.vector.tensor_add(
                                kT_rep[hh][0:64, ssl], ta[si, :], tsw[si, :]
                            )
                            nc.gpsimd.dma_start(
                                out=kT_rep[hh][64:128, ssl],
                                in_=kT_rep[hh][0:64, ssl],
                            )
                    else:
                        nc.vector.tensor_add(qT_sb[f - 2][:, ssl], ta, tsw)

                with (
                    tc.tile_pool(name="p1ps", bufs=2, space="PSUM") as p1ps,
                    tc.tile_pool(name="rtmp", bufs=3) as rtmp,
                ):
                    for group in ((0, 2), (3, 4), (5,)):
                        ts = {
                            f: [p1ps.tile([P, SCH], F32, tag=f"qkv{pr}",
                                          name=f"qkv{pr}")
                                for pr in range(2)]
                            for f in group
                        }
                        for h in range(H // P):
                            for f in group:
                                for pr in range(2):
                                    for hf in range(2):
                                        cs = slice(hf * 512, hf * 512 + 512)
                                        nc.tensor.matmul(
                                            ts[f][pr][:, cs],
                                            lhsT=wq_all[:, h, f * P:(f + 1) * P],
                                            rhs=xts[pr][:, h, cs],
                                            start=(h == 0),
                                            stop=(h == H // P - 1),
                                        )
                        for f in group:
                            for pr in range(2):
                                rope_feature(ts[f][pr], f, pr,
                                             slice(pr * SCH, (pr + 1) * SCH))
                with (
                    tc.tile_pool(name="p1v", bufs=1, space="PSUM") as p1v,
                    tc.tile_pool(name="p1vt", bufs=2, space="PSUM") as p1vt,
                    tc.tile_pool(name="vtt", bufs=2) as vtt,
                ):
                    f = 1
                    ts = [p1v.tile([P, SCH], F32, tag=f"v{pr}", name=f"v{pr}")
                          for pr in range(2)]
                    for h in range(H // P):
                        for pr in range(2):
                            for hf in range(2):
                                cs = slice(hf * 512, hf * 512 + 512)
                                nc.tensor.matmul(
                                    ts[pr][:, cs],
                                    lhsT=wq_all[:, h, f * P:(f + 1) * P],
                                    rhs=xts[pr][:, h, cs],
                                    start=(h == 0),
                                    stop=(h == H // P - 1),
                                )
                    for pr in range(2):
                        vt = vtt.tile([P, SCH], F16, tag="vt")
                        nc.vector.tensor_copy(out=vt, in_=ts[pr])
                        for hv in range(NKVL):
                            for j in range(SCH // P):
                                kb = (pr * SCH + j * P) // P
                                pvtr = p1vt.tile([P, HD], F16, tag="vtp", name="vtp")
                                nc.tensor.transpose(
                                    pvtr,
                                    vt[hv * HD:(hv + 1) * HD, j * P:(j + 1) * P],
                                    idn_sb[hv * HD:(hv + 1) * HD, :],
                                )
                                nc.vector.tensor_copy(
                                    out=v_sb[hv][kb][:, 64:128], in_=pvtr
                                )

            # ---------------- Phase 2 + 3: attention passes + o-projection
            OCH = 512
            with (
                tc.tile_pool(name="probs", bufs=6) as prp,
                tc.tile_pool(name="dvt", bufs=2) as dvt,
                tc.tile_pool(name="osb", bufs=2) as osb,
            ):
                osb_t = {}

                def emit_opiece(qb, pair, on_act):
                    if pair == 0:
                        osb_t[qb] = osb.tile([P, 4, OCH], F16, tag="ot", name="ot")
                    pos = [pools["p3"].tile([P, OCH], F32, tag=f"po{i}", name=f"po{i}")
                           for i in range(2)]
                    for c in range(4):
                        for i in range(2):
                            nch = pair * 2 + i
                            nc.tensor.matmul(
                                pos[i],
                                lhsT=attnT_sb[c][:, qb * P:(qb + 1) * P],
                                rhs=wo_all[:, c, nch * OCH:(nch + 1) * OCH],
                                start=(c == 0),
                                stop=(c == 3),
                            )
                    for i in range(2):
                        dst = osb_t[qb][:, pair * 2 + i, :]
                        if on_act:
                            nc.scalar.copy(out=dst, in_=pos[i])
                        else:
                            nc.vector.tensor_copy(out=dst, in_=pos[i])
                    if pair == 1:
                        nc.sync.dma_start(
                            out=out[qb * P:(qb + 1) * P, :], in_=osb_t[qb]
                        )

                def emit_head_pass(hl, pas, fillers=()):
                    fillers = list(fillers)
                    qc, qoff = hl // 2, (hl % 2) * HD
                    hv = hl // GROUPS
                    kTh = kT_rep[hv][qoff:qoff + HD, :]
                    qTh = qT_sb[qc][qoff:qoff + HD, :]
                    qlo = pas * 1024
                    pvt = pools["pv"].tile(
                        [P, 1024], F32,
                        tag=f"pv{hl % 2 if pas == 0 else 0}", name="pv")
                    jobs = []
                    for kb in range((pas + 1) * 8):
                        q0 = max(kb * P, qlo)
                        jobs.append((kb, q0, qlo + 1024 - q0))

                    def emit_qk(job):
                        kb, q0, W = job
                        sc = pools["sc"].tile([P, 1024], F32, tag="sc", name="sc")
                        mm0 = 0
                        while mm0 < W:
                            mw = min(512, W - mm0)
                            nc.tensor.matmul(
                                sc[:, mm0:mm0 + mw],
                                lhsT=kTh[:, kb * P:(kb + 1) * P],
                                rhs=qTh[:, q0 + mm0:q0 + mm0 + mw],
                                start=True,
                                stop=True,
                            )
                            mm0 += mw
                        return sc

                    pend = [emit_qk(jobs[0])]
                    for idx, job in enumerate(jobs):
                        kb, q0, W = job
                        sc = pend.pop(0)
                        if idx + 1 < len(jobs):
                            pend.append(emit_qk(jobs[idx + 1]))
                        col0 = q0 - qlo
                        pt = prp.tile([P, 1024], F16, tag="pt", name="pt")
                        nc.scalar.activation(
                            out=pt[:, col0:col0 + W],
                            in_=sc[:, 0:W],
                            func=mybir.ActivationFunctionType.Exp,
                            scale=0.125,
                        )
                        if kb * P >= qlo:
                            nc.vector.tensor_mul(
                                pt[:, col0:col0 + P],
                                pt[:, col0:col0 + P],
                                tri_sb,
                            )
                        for g in range(2):
                            glo, ghi = g * 512, (g + 1) * 512
                            if ghi <= col0:
                                continue
                            lo = max(glo, col0)
                            nc.tensor.matmul(
                                pvt[:, lo:ghi],
                                lhsT=v_sb[hv][kb][:, 0:P],
                                rhs=pt[:, lo:ghi],
                                start=(kb == 0),
                                stop=(kb == (qlo + ghi) // P - 1),
                            )
                        if fillers and idx % 8 == 7:
                            qb, pair, gp = fillers.pop(0)
                            emit_opiece(qb, pair, gp)
                    for qb, pair, gp in fillers:
                        emit_opiece(qb, pair, gp)
                    # normalize: attnT[d, q] = pv[64+d, q] * (1 / pv[0, q])
                    rc = dvt.tile([1, 1024], F32, tag="rc", name="rc")
                    nc.vector.reciprocal_approx_fast(out=rc, in_=pvt[0:1, :])
                    rcb = dvt.tile([HD, 1024], F32, tag="rcb", name="rcb")
                    nc.gpsimd.partition_broadcast(rcb, rc, channels=HD)
                    osl = slice(qlo, qlo + 1024)
                    nc.vector.tensor_mul(
                        attnT_sb[qc][qoff:qoff + HD, osl],
                        pvt[64:128, :],
                        rcb,
                    )

                pools = {}
                with (
                    tc.tile_pool(name="p2scA", bufs=2, space="PSUM") as scA,
                    tc.tile_pool(name="p2pvA", bufs=1, space="PSUM") as pvA,
                ):
                    pools["sc"] = scA
                    pools["pv"] = pvA
                    for hl in range(NHL):
                        emit_head_pass(hl, 0)
                with (
                    tc.tile_pool(name="p2scB", bufs=2, space="PSUM") as scB,
                    tc.tile_pool(name="p2pvB", bufs=1, space="PSUM") as pvB,
                    tc.tile_pool(name="p3ps", bufs=1, space="PSUM") as p3,
                ):
                    pools["sc"] = scB
                    pools["pv"] = pvB
                    pools["p3"] = p3
                    for hl in range(NHL):
                        # o-proj pieces for q-blocks 0..7 (pass-A attnT)
                        fillers = [(hl, pair, False) for pair in range(2)]
                        emit_head_pass(hl, 1, fillers)
                    for qb in range(8, S // P):
                        for pair in range(2):
                            emit_opiece(qb, pair, False)

    nc.compile()
    return nc


def _host_tables():
    # quadrant layout: within each 32-partition quadrant, positions 0:16 are
    # even (t1) slots and 16:32 odd (t2) slots; freq index = q16*16 + i.
    inv = (1.0 / ROPE_BASE ** (np.arange(0, HD, 2) / HD)).astype(np.float64)  # [32]
    ang = np.arange(S, dtype=np.float64)[:, None] * inv[None, :]  # [S, 32]
    cos32 = np.cos(ang).T  # [32, S] rows = freq index
    sin32 = np.sin(ang).T
    cos64 = np.empty((64, S))
    sin64 = np.empty((64, S))
    for q in range(2):
        fr = slice(q * 16, q * 16 + 16)
        cos64[q * 32:q * 32 + 16] = cos32[fr]
        cos64[q * 32 + 16:q * 32 + 32] = cos32[fr]
        sin64[q * 32:q * 32 + 16] = -sin32[fr]      # even slots: -sin
        sin64[q * 32 + 16:q * 32 + 32] = sin32[fr]  # odd slots: +sin
    cosx = np.tile(cos64, (2, 1)).astype(np.float16)  # [128, S]
    sinx = np.tile(sin64, (2, 1)).astype(np.float16)
    tri = (np.arange(P)[None, :] >= np.arange(P)[:, None]).astype(np.float16)
    idn = np.concatenate([np.eye(64, dtype=np.float16)] * 2, axis=0)  # [128, 64]
    return cosx, sinx, tri, idn


# per-head column permutation: quadrant q holds dims 32q..32q+31; evens first.
_PERM = np.concatenate(
    [np.concatenate([np.arange(32 * q, 32 * q + 32, 2),
                     np.arange(32 * q + 1, 32 * q + 32, 2)]) for q in range(2)]
)


def make_in_maps(x, w_qkv, w_o):
    """Build the 8 per-core input maps from full inputs."""
    cosx, sinx, tri, idn = _host_tables()
    in_maps = []
    for c in range(8):
        b, g = c // 4, c % 4
        xTc = np.ascontiguousarray(x[b].T).astype(np.float16)
        cols = []
        for kv in range(NKVL * g, NKVL * (g + 1)):
            cols.append(H + kv * HD + _PERM)
        kcols = np.concatenate(cols)
        cols = []
        for kv in range(NKVL * g, NKVL * (g + 1)):
            cols.append(H + NKV * HD + kv * HD + np.arange(HD))
        vcols = np.concatenate(cols)
        cols = []
        for hq in range(NHL * g, NHL * (g + 1)):
            cols.append(hq * HD + _PERM)
        qcols = np.concatenate(cols)
        wc = np.concatenate(
            [w_qkv[:, kcols], w_qkv[:, vcols], w_qkv[:, qcols]], axis=1
        ).astype(np.float16)
        woc = w_o[FQ * g:FQ * (g + 1), :].astype(np.float16)
        in_maps.append(
            {
                "xT": xTc,
                "wqkv": wc,
                "wo": woc,
                "cosx": cosx,
                "sinx": sinx,
                "tri": tri,
                "idn": idn,
            }
        )
    return in_maps


_NC = None


def get_nc():
    global _NC
    if _NC is None:
        _NC = build_bass()
    return _NC


def kernel(x, mask, w_qkv, w_o):
    x = np.asarray(x)
    w_qkv = np.asarray(w_qkv)
    w_o = np.asarray(w_o)
    nc = get_nc()
    in_maps = make_in_maps(x, w_qkv, w_o)
    res = run_bass_kernel_spmd(nc, in_maps, core_ids=list(range(8)))
    out = np.zeros((2, S, H), dtype=np.float32)
    for c in range(8):
        out[c // 4] += res.results[c]["out"].astype(np.float32)
    return out

